# revision 3
# baseline (speedup 1.0000x reference)
"""Trainium2 Bass kernel for nn_CAM_Module (channel-attention module).

Math per batch n (N = B*D = 128 independent problems):
    V = x[b, :, d, :, :].reshape(C, S)          # C=128, S=4096
    G = V @ V.T                                  # (C, C) Gram / energy
    A = softmax(rowmin(G) - ... ) == softmax(-G) row-wise (stabilized)
    out_n = (gamma * A + I) @ V                  # == gamma*(A@V) + V

Sharding: data-parallel over n across 8 NeuronCores (16 n per core).
Each core: DMA V in (c-part, s-free) layout; build U = V^T via PE
transposes (32x 128x128 chunks); G = sum_k Uk^T @ Uk (fp32 matmuls);
row softmax via DVE reduce + ACT exp (accumulating Z); fold gamma and
+I into the attention matrix; transpose it; stage-2 matmuls in
float32r (full PE rate at N=512) produce the final output directly.
"""

import numpy as np
from contextlib import ExitStack

import concourse.bass as bass
import concourse.tile as tile
from concourse import bacc, mybir
from concourse.bass_utils import run_bass_kernel_spmd

B, C, D, H, W = 4, 128, 32, 64, 64
S = H * W                  # 4096
N_TOTAL = B * D            # 128
N_CORES = 8
N_PER_CORE = N_TOTAL // N_CORES   # 16

FP = mybir.dt.float32
FPR = mybir.dt.float32r
AF = mybir.ActivationFunctionType
ALU = mybir.AxisListType

_CACHE = {}


def build_program(n_per_core=N_PER_CORE):
    key = n_per_core
    if key in _CACHE:
        return _CACHE[key]

    nc = bacc.Bacc(
        "TRN2", target_bir_lowering=False, debug=False, num_devices=N_CORES
    )
    # xs declared float32r: raw fp32 bits; PE rounds operands on read, so the
    # stage-2 matmuls can consume V directly at full PE rate (1 cyc/row at
    # N=512). fp32 uses go through .bitcast(FP).
    xs = nc.dram_tensor("xs", [n_per_core, C, S], FPR, kind="ExternalInput").ap()
    gamma_b = nc.dram_tensor("gamma_b", [C, 1], FP, kind="ExternalInput").ap()
    ident = nc.dram_tensor("ident", [C, C], FP, kind="ExternalInput").ap()
    out = nc.dram_tensor("out", [n_per_core, C, S], FP, kind="ExternalOutput").ap()

    NCHUNK = S // C            # 32 transpose chunks per n
    NJ = S // 512              # 8 512-wide column groups

    with tile.TileContext(nc) as tc, ExitStack() as ctx:
        const_pool = ctx.enter_context(tc.tile_pool(name="const", bufs=1))
        v_pool = ctx.enter_context(tc.tile_pool(name="v", bufs=3))
        u_pool = ctx.enter_context(tc.tile_pool(name="u", bufs=2))
        small_pool = ctx.enter_context(tc.tile_pool(name="small", bufs=3))
        osb_pool = ctx.enter_context(tc.tile_pool(name="osb", bufs=2))
        tr_ps_pool = ctx.enter_context(tc.tile_pool(name="trps", bufs=2, space="PSUM"))
        g_ps_pool = ctx.enter_context(tc.tile_pool(name="gps", bufs=2, space="PSUM"))
        o_ps_pool = ctx.enter_context(tc.tile_pool(name="ops", bufs=2, space="PSUM"))

        id_sb = const_pool.tile([C, C], FP)
        nc.sync.dma_start(id_sb[:], ident[:])
        gam_sb = const_pool.tile([C, 1], FP)
        nc.sync.dma_start(gam_sb[:], gamma_b[:])

        for n in range(n_per_core):
            v_sb = v_pool.tile([C, S], FPR)
            nc.sync.dma_start(v_sb[:], xs[n])

            # U = V^T, stored as 32 chunks: u_sb[p, 128k + c] = V[c, 128k + p]
            u_sb = u_pool.tile([C, S], FP)
            for j in range(NCHUNK // 4):
                t_ps = tr_ps_pool.tile([C, 512], FP)
                for q in range(4):
                    k = 4 * j + q
                    nc.tensor.transpose(
                        t_ps[:, 128 * q : 128 * (q + 1)],
                        v_sb[:, 128 * k : 128 * (k + 1)].bitcast(FP),
                        id_sb[:],
                    )
                nc.scalar.copy(u_sb[:, 512 * j : 512 * (j + 1)], t_ps[:])

            # G = V @ V^T = sum_k Uk^T @ Uk  (fp32, accumulate in PSUM)
            g_ps = g_ps_pool.tile([C, C], FP)
            for k in range(NCHUNK):
                ck = u_sb[:, 128 * k : 128 * (k + 1)]
                nc.tensor.matmul(
                    g_ps[:], ck, ck, start=(k == 0), stop=(k == NCHUNK - 1)
                )

            # Row softmax of (rowmin - G): numer = exp(rowmin - G), Z = sum
            rmin = small_pool.tile([C, 1], FP)
            nc.vector.tensor_reduce(
                rmin[:], g_ps[:], axis=ALU.X, op=mybir.AluOpType.min
            )
            numer = small_pool.tile([C, C], FP)
            zsum = small_pool.tile([C, 1], FP)
            nc.scalar.activation(
                numer[:], g_ps[:], AF.Exp,
                bias=rmin[:], scale=-1.0, accum_out=zsum[:],
            )
            zinv = small_pool.tile([C, 1], FP)
            nc.vector.reciprocal(zinv[:], zsum[:])
            gz = small_pool.tile([C, 1], FP)
            nc.vector.tensor_mul(gz[:], zinv[:], gam_sb[:])

            # Abar = gamma * A + I ; then transpose for use as stage-2 lhsT
            abar = small_pool.tile([C, C], FP)
            nc.vector.tensor_scalar_mul(abar[:], numer[:], gz[:])
            nc.vector.tensor_add(abar[:], abar[:], id_sb[:])
            at_ps = g_ps_pool.tile([C, C], FP)
            nc.tensor.transpose(at_ps[:], abar[:], id_sb[:])
            abt = small_pool.tile([C, C], FPR)
            nc.vector.tensor_copy(abt[:], at_ps[:])

            # out_n = Abar @ V   (float32r matmuls, N=512)
            o_sb = osb_pool.tile([C, S], FP)
            for j in range(NJ):
                o_ps = o_ps_pool.tile([C, 512], FP)
                nc.tensor.matmul(
                    o_ps[:],
                    abt[:],
                    v_sb[:, 512 * j : 512 * (j + 1)],
                    start=True, stop=True,
                )
                nc.vector.tensor_copy(o_sb[:, 512 * j : 512 * (j + 1)], o_ps[:])
            nc.sync.dma_start(out[n], o_sb[:])

    nc.compile()
    _CACHE[key] = nc
    return nc


def make_in_maps(x, gamma, n_per_core=N_PER_CORE):
    """Shard full inputs into per-core input maps (data-parallel over B*D)."""
    x = np.asarray(x, dtype=np.float32)
    gamma = np.asarray(gamma, dtype=np.float32).reshape(-1)
    gamma_b = np.full((C, 1), gamma[0], dtype=np.float32)
    ident = np.eye(C, dtype=np.float32)
    # v[n=(b,d)][c,s] = x[b,c,d,s] ; core i takes n in [i*npc, (i+1)*npc)
    xt = np.ascontiguousarray(
        x.reshape(B, C, D, S).transpose(0, 2, 1, 3)
    ).reshape(N_TOTAL, C, S)
    in_maps = []
    for i in range(N_CORES):
        xs = np.ascontiguousarray(xt[i * n_per_core : (i + 1) * n_per_core])
        in_maps.append({"xs": xs, "gamma_b": gamma_b, "ident": ident})
    return in_maps


def run_on_cores(x, gamma, trace=False, **kw):
    nc = build_program()
    in_maps = make_in_maps(x, gamma)
    res = run_bass_kernel_spmd(
        nc, in_maps, core_ids=list(range(N_CORES)), trace=trace, **kw
    )
    return res


def assemble_output(results):
    parts = [results[i]["out"] for i in range(N_CORES)]
    full = np.concatenate(parts, axis=0)          # (B*D, C, S) contiguous
    # reference returns a raw reinterpret of contiguous (B, D, C, H, W)
    return full.reshape(B, C, D, H, W)


def kernel(x, gamma):
    res = run_on_cores(x, gamma, trace=False)
    return assemble_output(res.results)


# revision 4
# speedup vs baseline: 1.0921x; 1.0921x over previous
"""Trainium2 Bass kernel for nn_CAM_Module (channel-attention module).

Math per batch n (N = B*D = 128 independent problems):
    V = x[b, :, d, :, :].reshape(C, S)          # C=128, S=4096
    G = V @ V.T                                  # (C, C) Gram / energy
    A = softmax(-G) row-wise (stabilized with rowmin subtract)
    out_n = (gamma * A + I) @ V                  # == gamma*(A@V) + V

Sharding: data-parallel over n across 8 NeuronCores (16 n per core).

Per-core pipeline (software-pipelined by one n so PE never waits on the
softmax chain):
  - DMA V in (c-part, s-free) layout (fp32 bits, DRAM declared float32r)
  - 32x PE 128x128 transposes (fp32) -> PSUM; ACT copies cast to fp16 U
  - stage 1: G = sum_k Uk^T @ Uk in fp16 (PSUM fp32 accumulate)
  - softmax: DVE rowmin reduce; ACT exp(rowmin - G) with accumulated Z;
    DVE fused (numer * gamma/Z) + I; PE transpose -> abt (float32r)
  - stage 2 (deferred one n): out = Abar @ V, float32r matmuls N=512;
    DVE copies PSUM->SBUF; 2 output DMAs per n
"""

import numpy as np
from contextlib import ExitStack

import concourse.bass as bass
import concourse.tile as tile
from concourse import bacc, mybir
from concourse.bass_utils import run_bass_kernel_spmd

B, C, D, H, W = 4, 128, 32, 64, 64
S = H * W                  # 4096
N_TOTAL = B * D            # 128
N_CORES = 8
N_PER_CORE = N_TOTAL // N_CORES   # 16

FP = mybir.dt.float32
FPR = mybir.dt.float32r
FP16 = mybir.dt.float16
AF = mybir.ActivationFunctionType
AX = mybir.AxisListType
OP = mybir.AluOpType

_CACHE = {}

# tuning flags
STAGE1_DT = FP16        # fp16: ~81ns/chunk vs fp32 ~220ns/chunk
TR_DT = FP              # transpose dtype (fp32 exact; fp32r would be 1.5cyc/row)


def build_program(n_per_core=N_PER_CORE):
    key = n_per_core
    if key in _CACHE:
        return _CACHE[key]

    nc = bacc.Bacc(
        "TRN2", target_bir_lowering=False, debug=False, num_devices=N_CORES
    )
    # xs declared float32r: raw fp32 bits; PE rounds operands on read, so
    # stage-2 matmuls consume V directly at full PE rate (no cast pass).
    xs = nc.dram_tensor("xs", [n_per_core, C, S], FPR, kind="ExternalInput").ap()
    gamma_b = nc.dram_tensor("gamma_b", [C, 1], FP, kind="ExternalInput").ap()
    ident = nc.dram_tensor("ident", [C, C], FP, kind="ExternalInput").ap()
    out = nc.dram_tensor("out", [n_per_core, C, S], FP, kind="ExternalOutput").ap()

    NCHUNK = S // C            # 32 transpose chunks per n
    NJ = S // 512              # 8 512-wide column groups

    with tile.TileContext(nc) as tc, ExitStack() as ctx:
        const_pool = ctx.enter_context(tc.tile_pool(name="const", bufs=1))
        v_pool = ctx.enter_context(tc.tile_pool(name="v", bufs=4))
        u_pool = ctx.enter_context(tc.tile_pool(name="u", bufs=2))
        small_pool = ctx.enter_context(tc.tile_pool(name="small", bufs=3))
        osb_pool = ctx.enter_context(tc.tile_pool(name="osb", bufs=2))
        tr_ps_pool = ctx.enter_context(tc.tile_pool(name="trps", bufs=2, space="PSUM"))
        g_ps_pool = ctx.enter_context(tc.tile_pool(name="gps", bufs=2, space="PSUM"))
        at_ps_pool = ctx.enter_context(tc.tile_pool(name="atps", bufs=1, space="PSUM"))
        o_ps_pool = ctx.enter_context(tc.tile_pool(name="ops", bufs=3, space="PSUM"))

        id_sb = const_pool.tile([C, C], FP)
        nc.sync.dma_start(id_sb[:], ident[:])
        gam_sb = const_pool.tile([C, 1], FP)
        nc.sync.dma_start(gam_sb[:], gamma_b[:])

        prev = None  # (v_sb, abt) carried to next iteration for stage 2

        for n in range(n_per_core + 1):
            cur = None
            if n < n_per_core:
                v_sb = v_pool.tile([C, S], FPR)
                nc.sync.dma_start(v_sb[:], xs[n])

                # U = V^T in fp16: PE transposes (fp32) + ACT cast-copies
                u_sb = u_pool.tile([C, S], STAGE1_DT)
                for j in range(NCHUNK // 4):
                    t_ps = tr_ps_pool.tile([C, 512], TR_DT)
                    for q in range(4):
                        k = 4 * j + q
                        nc.tensor.transpose(
                            t_ps[:, 128 * q : 128 * (q + 1)],
                            v_sb[:, 128 * k : 128 * (k + 1)].bitcast(TR_DT),
                            id_sb[:].bitcast(TR_DT),
                        )
                    nc.scalar.copy(u_sb[:, 512 * j : 512 * (j + 1)], t_ps[:])

                # G = sum_k Uk^T @ Uk
                g_ps = g_ps_pool.tile([C, C], FP)
                for k in range(NCHUNK):
                    ck = u_sb[:, 128 * k : 128 * (k + 1)]
                    nc.tensor.matmul(
                        g_ps[:], ck, ck, start=(k == 0), stop=(k == NCHUNK - 1)
                    )

                # softmax chain (DVE/ACT): abar = (exp(rowmin-G)/Z)*gamma + I
                rmin = small_pool.tile([C, 1], FP)
                nc.vector.tensor_reduce(rmin[:], g_ps[:], axis=AX.X, op=OP.min)
                numer = small_pool.tile([C, C], FP)
                zsum = small_pool.tile([C, 1], FP)
                nc.scalar.activation(
                    numer[:], g_ps[:], AF.Exp,
                    bias=rmin[:], scale=-1.0, accum_out=zsum[:],
                )
                zinv = small_pool.tile([C, 1], FP)
                nc.vector.reciprocal(zinv[:], zsum[:])
                gz = small_pool.tile([C, 1], FP)
                nc.vector.tensor_mul(gz[:], zinv[:], gam_sb[:])
                abar = small_pool.tile([C, C], FP)
                nc.vector.scalar_tensor_tensor(
                    abar[:], numer[:], gz[:], id_sb[:], op0=OP.mult, op1=OP.add
                )
                cur = (v_sb, abar)

            if prev is not None:
                # stage 2 for previous n (PE busy with cur's transposes first,
                # so prev's abt transpose + matmuls never stall)
                pv_sb, pabar = prev
                at_ps = at_ps_pool.tile([C, C], FP)
                nc.tensor.transpose(at_ps[:], pabar[:], id_sb[:])
                abt = small_pool.tile([C, C], FPR)
                nc.vector.tensor_copy(abt[:], at_ps[:])

                o_sb = osb_pool.tile([C, S], FP)
                for j in range(NJ):
                    o_ps = o_ps_pool.tile([C, 512], FP)
                    nc.tensor.matmul(
                        o_ps[:],
                        abt[:],
                        pv_sb[:, 512 * j : 512 * (j + 1)],
                        start=True, stop=True,
                    )
                    nc.vector.tensor_copy(
                        o_sb[:, 512 * j : 512 * (j + 1)], o_ps[:]
                    )
                    if j == NJ // 2 - 1:
                        nc.sync.dma_start(
                            out[n - 1, :, : S // 2], o_sb[:, : S // 2]
                        )
                nc.sync.dma_start(out[n - 1, :, S // 2 :], o_sb[:, S // 2 :])

            prev = cur

    nc.compile()
    _CACHE[key] = nc
    return nc


def make_in_maps(x, gamma, n_per_core=N_PER_CORE):
    """Shard full inputs into per-core input maps (data-parallel over B*D)."""
    x = np.asarray(x, dtype=np.float32)
    gamma = np.asarray(gamma, dtype=np.float32).reshape(-1)
    gamma_b = np.full((C, 1), gamma[0], dtype=np.float32)
    ident = np.eye(C, dtype=np.float32)
    # v[n=(b,d)][c,s] = x[b,c,d,s] ; core i takes n in [i*npc, (i+1)*npc)
    xt = np.ascontiguousarray(
        x.reshape(B, C, D, S).transpose(0, 2, 1, 3)
    ).reshape(N_TOTAL, C, S)
    in_maps = []
    for i in range(N_CORES):
        xs = np.ascontiguousarray(xt[i * n_per_core : (i + 1) * n_per_core])
        in_maps.append({"xs": xs, "gamma_b": gamma_b, "ident": ident})
    return in_maps


def run_on_cores(x, gamma, trace=False, **kw):
    nc = build_program()
    in_maps = make_in_maps(x, gamma)
    res = run_bass_kernel_spmd(
        nc, in_maps, core_ids=list(range(N_CORES)), trace=trace, **kw
    )
    return res


def assemble_output(results):
    parts = [results[i]["out"] for i in range(N_CORES)]
    full = np.concatenate(parts, axis=0)          # (B*D, C, S) contiguous
    # reference returns a raw reinterpret of contiguous (B, D, C, H, W)
    return full.reshape(B, C, D, H, W)


def kernel(x, gamma):
    res = run_on_cores(x, gamma, trace=False)
    return assemble_output(res.results)


# revision 6
# speedup vs baseline: 1.1276x; 1.0324x over previous
"""Trainium2 Bass kernel for nn_CAM_Module (channel-attention module).

Math per batch n (N = B*D = 128 independent problems):
    V = x[b, :, d, :, :].reshape(C, S)          # C=128, S=4096
    G = V @ V.T                                  # (C, C) Gram / energy
    A = softmax(-G) row-wise (stabilized with rowmin subtract)
    out_n = (gamma * A + I) @ V                  # == gamma*(A@V) + V

Sharding: data-parallel over n across 8 NeuronCores (16 n per core).

Per-core pipeline (software-pipelined by one n so PE never waits on the
softmax chain):
  - DMA V in (c-part, s-free) layout (fp32 bits, DRAM declared float32r)
  - 32x PE 128x128 transposes (fp32) -> PSUM; ACT copies cast to fp16 U
  - stage 1: G = sum_k Uk^T @ Uk in fp16 (PSUM fp32 accumulate)
  - softmax: DVE rowmin reduce; ACT exp(rowmin - G) with accumulated Z;
    DVE fused (numer * gamma/Z) + I; PE transpose -> abt (float32r)
  - stage 2 (deferred one n): out = Abar @ V, float32r matmuls N=512;
    DVE copies PSUM->SBUF; 2 output DMAs per n
"""

import numpy as np
from contextlib import ExitStack

import concourse.bass as bass
import concourse.tile as tile
from concourse import bacc, mybir
from concourse.bass_utils import run_bass_kernel_spmd

B, C, D, H, W = 4, 128, 32, 64, 64
S = H * W                  # 4096
N_TOTAL = B * D            # 128
N_CORES = 8
N_PER_CORE = N_TOTAL // N_CORES   # 16

FP = mybir.dt.float32
FPR = mybir.dt.float32r
FP16 = mybir.dt.float16
AF = mybir.ActivationFunctionType
AX = mybir.AxisListType
OP = mybir.AluOpType

_CACHE = {}

# tuning flags
STAGE1_DT = FP16        # fp16: ~81ns/chunk vs fp32 ~220ns/chunk
TR_DT = FP              # transpose dtype (fp32 exact; fp32r would be 1.5cyc/row)


def build_program(n_per_core=N_PER_CORE):
    key = n_per_core
    if key in _CACHE:
        return _CACHE[key]

    nc = bacc.Bacc(
        "TRN2", target_bir_lowering=False, debug=False, num_devices=N_CORES
    )
    # xs declared float32r: raw fp32 bits; PE rounds operands on read, so
    # stage-2 matmuls consume V directly at full PE rate (no cast pass).
    xs = nc.dram_tensor("xs", [n_per_core, C, S], FPR, kind="ExternalInput").ap()
    gamma_b = nc.dram_tensor("gamma_b", [C, 1], FP, kind="ExternalInput").ap()
    ident = nc.dram_tensor("ident", [C, C], FP, kind="ExternalInput").ap()
    out = nc.dram_tensor("out", [n_per_core, C, S], FP, kind="ExternalOutput").ap()

    NCHUNK = S // C            # 32 transpose chunks per n
    NJ = S // 512              # 8 512-wide column groups

    with tile.TileContext(nc) as tc, ExitStack() as ctx:
        const_pool = ctx.enter_context(tc.tile_pool(name="const", bufs=1))
        v_pool = ctx.enter_context(tc.tile_pool(name="v", bufs=4))
        u_pool = ctx.enter_context(tc.tile_pool(name="u", bufs=2))
        small_pool = ctx.enter_context(tc.tile_pool(name="small", bufs=3))
        osb_pool = ctx.enter_context(tc.tile_pool(name="osb", bufs=2))
        tr_ps_pool = ctx.enter_context(tc.tile_pool(name="trps", bufs=3, space="PSUM"))
        g_ps_pool = ctx.enter_context(tc.tile_pool(name="gps", bufs=2, space="PSUM"))
        at_ps_pool = ctx.enter_context(tc.tile_pool(name="atps", bufs=1, space="PSUM"))
        o_ps_pool = ctx.enter_context(tc.tile_pool(name="ops", bufs=2, space="PSUM"))

        id_sb = const_pool.tile([C, C], FP)
        nc.sync.dma_start(id_sb[:], ident[:])
        gam_sb = const_pool.tile([C, 1], FP)
        nc.sync.dma_start(gam_sb[:], gamma_b[:])

        def copy512(dst, src, idx):
            # alternate big PSUM->SBUF copies across ACT and DVE
            if idx % 2 == 0:
                nc.scalar.copy(dst, src)
            else:
                nc.vector.tensor_copy(dst, src)

        prev = None  # (v_sb, abar) carried to next iteration for stage 2

        for n in range(n_per_core + 1):
            cur = None
            if n < n_per_core:
                v_sb = v_pool.tile([C, S], FPR)
                nc.sync.dma_start(v_sb[:], xs[n])

                # U = V^T in fp16: PE transposes (fp32) + cast-copies
                u_sb = u_pool.tile([C, S], STAGE1_DT)
                for j in range(NCHUNK // 4):
                    t_ps = tr_ps_pool.tile([C, 512], TR_DT)
                    for q in range(4):
                        k = 4 * j + q
                        nc.tensor.transpose(
                            t_ps[:, 128 * q : 128 * (q + 1)],
                            v_sb[:, 128 * k : 128 * (k + 1)].bitcast(TR_DT),
                            id_sb[:].bitcast(TR_DT),
                        )
                    copy512(u_sb[:, 512 * j : 512 * (j + 1)], t_ps[:], j)

            if prev is not None:
                # stage 2 for previous n, emitted between cur's transposes and
                # cur's stage-1 matmuls: PE fills the u-copy latency with it.
                pv_sb, pabar = prev
                at_ps = at_ps_pool.tile([C, C], FP)
                nc.tensor.transpose(at_ps[:], pabar[:], id_sb[:])
                abt = small_pool.tile([C, C], FPR)
                nc.vector.tensor_copy(abt[:], at_ps[:])

                o_sb = osb_pool.tile([C, S], FP)
                for j in range(NJ):
                    o_ps = o_ps_pool.tile([C, 512], FP)
                    nc.tensor.matmul(
                        o_ps[:],
                        abt[:],
                        pv_sb[:, 512 * j : 512 * (j + 1)],
                        start=True, stop=True,
                    )
                    copy512(o_sb[:, 512 * j : 512 * (j + 1)], o_ps[:], j + 1)
                    if j == NJ // 2 - 1:
                        nc.sync.dma_start(
                            out[n - 1, :, : S // 2], o_sb[:, : S // 2]
                        )
                nc.sync.dma_start(out[n - 1, :, S // 2 :], o_sb[:, S // 2 :])

            if n < n_per_core:
                # G = sum_k Uk^T @ Uk
                g_ps = g_ps_pool.tile([C, C], FP)
                for k in range(NCHUNK):
                    ck = u_sb[:, 128 * k : 128 * (k + 1)]
                    nc.tensor.matmul(
                        g_ps[:], ck, ck, start=(k == 0), stop=(k == NCHUNK - 1)
                    )

                # softmax chain (DVE/ACT): abar = (exp(rowmin-G)/Z)*gamma + I
                rmin = small_pool.tile([C, 1], FP)
                nc.vector.tensor_reduce(rmin[:], g_ps[:], axis=AX.X, op=OP.min)
                numer = small_pool.tile([C, C], FP)
                zsum = small_pool.tile([C, 1], FP)
                nc.scalar.activation(
                    numer[:], g_ps[:], AF.Exp,
                    bias=rmin[:], scale=-1.0, accum_out=zsum[:],
                )
                zinv = small_pool.tile([C, 1], FP)
                nc.vector.reciprocal(zinv[:], zsum[:])
                gz = small_pool.tile([C, 1], FP)
                nc.vector.tensor_mul(gz[:], zinv[:], gam_sb[:])
                abar = small_pool.tile([C, C], FP)
                nc.vector.scalar_tensor_tensor(
                    abar[:], numer[:], gz[:], id_sb[:], op0=OP.mult, op1=OP.add
                )
                cur = (v_sb, abar)

            prev = cur

    nc.compile()
    _CACHE[key] = nc
    return nc


def make_in_maps(x, gamma, n_per_core=N_PER_CORE):
    """Shard full inputs into per-core input maps (data-parallel over B*D)."""
    x = np.asarray(x, dtype=np.float32)
    gamma = np.asarray(gamma, dtype=np.float32).reshape(-1)
    gamma_b = np.full((C, 1), gamma[0], dtype=np.float32)
    ident = np.eye(C, dtype=np.float32)
    # v[n=(b,d)][c,s] = x[b,c,d,s] ; core i takes n in [i*npc, (i+1)*npc)
    xt = np.ascontiguousarray(
        x.reshape(B, C, D, S).transpose(0, 2, 1, 3)
    ).reshape(N_TOTAL, C, S)
    in_maps = []
    for i in range(N_CORES):
        xs = np.ascontiguousarray(xt[i * n_per_core : (i + 1) * n_per_core])
        in_maps.append({"xs": xs, "gamma_b": gamma_b, "ident": ident})
    return in_maps


def run_on_cores(x, gamma, trace=False, **kw):
    nc = build_program()
    in_maps = make_in_maps(x, gamma)
    res = run_bass_kernel_spmd(
        nc, in_maps, core_ids=list(range(N_CORES)), trace=trace, **kw
    )
    return res


def assemble_output(results):
    parts = [results[i]["out"] for i in range(N_CORES)]
    full = np.concatenate(parts, axis=0)          # (B*D, C, S) contiguous
    # reference returns a raw reinterpret of contiguous (B, D, C, H, W)
    return full.reshape(B, C, D, H, W)


def kernel(x, gamma):
    res = run_on_cores(x, gamma, trace=False)
    return assemble_output(res.results)
